# revision 27
# baseline (speedup 1.0000x reference)
"""Trainium2 Bass kernel for BanditSwitchRNN (mixture-of-experts GRU).

Sharding: data-parallel over batch B=1024 -> 8 cores x 128 batch each.
Per-core state h kept transposed in SBUF as [H=128 partitions, 128 batch cols].

Per-step math (expert selection is one-hot over K=4, known from x alone):
  - selection folded into inputs on host: xm[t] = masked x (+ mask rows for
    biases), mrep[t] = per-expert masks replicated over the 128 partitions.
  - h-side: hm[:, k*128:(k+1)*128] = h * mask_k  (4 DVE mults), then per gate
    4 accumulating matmuls lhsT=Wh_g[k].T, rhs=hm_k into one PSUM bank, with
    the gate's x-side matmul (lhsT=wxm block, rhs=xm[t]) opening the bank.
  - gates: r,z = sigmoid (ACT), n = tanh(Pnx + r*Pnh), h' = n + z*(h-n).
  - logits deferred: every h_t stored in a big SBUF buffer; one bulk matmul
    pass with Wout.T after the scan.
All matmul operands bf16, PSUM accumulation fp32 (validated 5.6e-3 rel err).
"""
import sys, os
sys.path.insert(0, '/opt/trn_rl_repo')
from contextlib import ExitStack

import numpy as np
import ml_dtypes

import concourse.bass as bass
import concourse.bacc as bacc
import concourse.mybir as mybir
import concourse.tile as tile
from concourse.bass_utils import run_bass_kernel_spmd

B, T, D, H, K, A = 1024, 256, 3, 128, 4, 2
NCORES = 8
BL = B // NCORES            # 128 batch per core
XR = 16                     # xm contraction rows: 12 masked-x + 4 mask (bias)
BF16 = mybir.dt.bfloat16
F32 = mybir.dt.float32
nbf = ml_dtypes.bfloat16
SIG = mybir.ActivationFunctionType.Sigmoid
TANH = mybir.ActivationFunctionType.Tanh

_CACHE = {}
LAST_RESULTS = None


def _build():
    nc = bacc.Bacc(None)
    whT = nc.declare_dram_parameter("whT", [3 * K, H, H], BF16, isOutput=False)
    wxm = nc.declare_dram_parameter("wxm", [4, XR, H], BF16, isOutput=False)
    wout = nc.declare_dram_parameter("wout", [H, A], BF16, isOutput=False)
    xm = nc.declare_dram_parameter("xm", [T, XR, BL], BF16, isOutput=False)
    mrep = nc.declare_dram_parameter("mrep", [T, H, K * BL], BF16, isOutput=False)
    lg = nc.declare_dram_parameter("lg", [A, T * BL], BF16, isOutput=True)
    hT = nc.declare_dram_parameter("hT", [H, BL], BF16, isOutput=True)

    with tile.TileContext(nc) as tc, ExitStack() as ctx:
        consts = ctx.enter_context(tc.tile_pool(name="consts", bufs=1))
        hallp = ctx.enter_context(tc.tile_pool(name="hall", bufs=1))

        whT_sb = []
        for i in range(3 * K):
            t_ = consts.tile([H, H], BF16, tag=f"whT{i}")
            nc.gpsimd.dma_start(t_[:], whT[i])
            whT_sb.append(t_)
        wxm_sb = []
        for i in range(4):
            t_ = consts.tile([XR, H], BF16, tag=f"wxm{i}")
            nc.gpsimd.dma_start(t_[:], wxm[i])
            wxm_sb.append(t_)
        wout_sb = consts.tile([H, A], BF16, tag="wout")
        nc.gpsimd.dma_start(wout_sb[:], wout[:])

        # Hall slot s holds h after step s-1 (slot 0 = h0 = 0)
        Hall = hallp.tile([H, (T + 1) * BL], BF16, tag="Hall")
        nc.vector.memset(Hall[:, 0:BL], 0.0)

        with (
            tc.tile_pool(name="stream", bufs=6) as streamp,
            tc.tile_pool(name="work", bufs=3) as workp,
            tc.tile_pool(name="psum", bufs=2, space="PSUM") as psump,
        ):
            hm = workp.tile([H, K * BL], BF16, tag="hm", name="hm")
            nc.vector.memset(hm[:], 0.0)
            for t in range(T):
                xm_t = streamp.tile([XR, BL], BF16, tag="xm")
                nc.gpsimd.dma_start(xm_t[:], xm[t])
                mr_t = streamp.tile([H, K * BL], BF16, tag="mrep")
                nc.gpsimd.dma_start(mr_t[:], mrep[t])

                Pr = psump.tile([H, BL], F32, tag="Pr")
                Pz = psump.tile([H, BL], F32, tag="Pz")
                Pnx = psump.tile([H, BL], F32, tag="Pnx")
                Pnh = psump.tile([H, BL], F32, tag="Pnh")

                # x-matmuls open each gate's accumulation group
                for q, (P, has_h) in enumerate(
                        ((Pr, True), (Pz, True), (Pnx, False), (Pnh, True))):
                    nc.tensor.matmul(P, wxm_sb[q][:], xm_t[:],
                                     start=True, stop=not has_h)
                # h-side: r, z first (sigmoid heads the tail), then nh
                for P, g in ((Pr, 0), (Pz, 1), (Pnh, 2)):
                    for k in range(K):
                        nc.tensor.matmul(P, whT_sb[g * K + k][:],
                                         hm[:, k * BL:(k + 1) * BL],
                                         start=False, stop=(k == K - 1))

                r_sb = workp.tile([H, BL], BF16, tag="r")
                nc.scalar.activation(r_sb[:], Pr[:], SIG)
                z_sb = workp.tile([H, BL], BF16, tag="z")
                nc.scalar.activation(z_sb[:], Pz[:], SIG)
                t1 = workp.tile([H, BL], F32, tag="t1")
                nc.vector.tensor_mul(t1[:], r_sb[:], Pnh[:])
                t2 = workp.tile([H, BL], F32, tag="t2")
                nc.vector.tensor_add(t2[:], t1[:], Pnx[:])
                n_sb = workp.tile([H, BL], BF16, tag="n")
                nc.scalar.activation(n_sb[:], t2[:], TANH)

                h_prev = Hall[:, t * BL:(t + 1) * BL]
                h_new = Hall[:, (t + 1) * BL:(t + 2) * BL]
                d_sb = workp.tile([H, BL], BF16, tag="d")
                nc.vector.tensor_sub(d_sb[:], h_prev, n_sb[:])
                e_sb = workp.tile([H, BL], BF16, tag="e")
                nc.vector.tensor_mul(e_sb[:], z_sb[:], d_sb[:])
                nc.vector.tensor_add(h_new, n_sb[:], e_sb[:])

                hm = workp.tile([H, K * BL], BF16, tag="hm", name="hm")
                h_bc = h_new.rearrange("p (o b) -> p o b", o=1)
                h_bc = h_bc.broadcast_to([H, K, BL])
                nc.vector.tensor_mul(
                    hm[:].rearrange("p (k b) -> p k b", k=K), h_bc,
                    mr_t[:].rearrange("p (k b) -> p k b", k=K))

        # logits end-pass: lg[a, t*BL+b] = sum_i Wout[a,i] * Hall[i, slot t+1]
        with (
            tc.tile_pool(name="lgp", bufs=4, space="PSUM") as lgp,
            tc.tile_pool(name="lgs", bufs=4) as lgsp,
        ):
            CH = 512
            for ci in range((T * BL) // CH):
                ps = lgp.tile([A, CH], F32, tag="lgps")
                nc.tensor.matmul(ps[:], wout_sb[:],
                                 Hall[:, BL + ci * CH:BL + (ci + 1) * CH],
                                 start=True, stop=True)
                st = lgsp.tile([A, CH], BF16, tag="lgsb")
                if ci % 2 == 0:
                    nc.scalar.copy(st[:], ps[:])
                else:
                    nc.vector.tensor_copy(st[:], ps[:])
                nc.gpsimd.dma_start(lg[:, ci * CH:(ci + 1) * CH], st[:])
            nc.gpsimd.dma_start(hT[:], Hall[:, T * BL:(T + 1) * BL])
    if not nc.is_finalized():
        nc.finalize()
    return nc


def _prep_inputs(inputs):
    x = np.asarray(inputs['x'], np.float32)
    a_sel = np.argmax(x[:, :, :2], axis=-1)
    r_sel = (x[:, :, 2] > 0.5).astype(np.int64)
    k_sel = a_sel + 2 * r_sel                                   # [B, T]
    mask = (k_sel[None] == np.arange(K)[:, None, None]).astype(np.float32)

    Wh = [np.asarray(inputs['Wh' + g], np.float32) for g in 'rzn']
    Wx = [np.asarray(inputs['Wx' + g], np.float32) for g in 'rzn']
    bx = [np.asarray(inputs['bx' + g], np.float32) for g in 'rzn']
    bh = {g: np.asarray(inputs['b' + g + 'h'], np.float32) for g in 'rzn'}

    whT = np.empty((3 * K, H, H), np.float32)
    for g in range(3):
        for k in range(K):
            whT[g * K + k] = Wh[g][k].T
    wxm = np.zeros((4, XR, H), np.float32)
    for bi, (Wxg, bsum) in enumerate((
            (Wx[0], bx[0] + bh['r']), (Wx[1], bx[1] + bh['z']),
            (Wx[2], bx[2]), (None, bh['n']))):
        if Wxg is not None:
            for k in range(K):
                for d in range(D):
                    wxm[bi, k * D + d] = Wxg[k, :, d]
        for k in range(K):
            wxm[bi, 3 * K + k] = bsum[k]
    wout = np.asarray(inputs['Wout'], np.float32).T             # [H, A]

    shared = {'whT': whT.astype(nbf), 'wxm': wxm.astype(nbf),
              'wout': wout.astype(nbf)}
    in_maps = []
    for c in range(NCORES):
        bc = slice(c * BL, (c + 1) * BL)
        mc = mask[:, bc, :]                                     # [K, BL, T]
        xc = x[bc]                                              # [BL, T, D]
        xm_c = np.zeros((T, XR, BL), np.float32)
        for k in range(K):
            for d in range(D):
                xm_c[:, k * D + d, :] = (mc[k] * xc[:, :, d]).T
            xm_c[:, 3 * K + k, :] = mc[k].T
        # mrep[t] = mask of step t+1 (hm built at end of step t feeds t+1),
        # replicated over the j partitions; last slot unused -> zeros.
        m_next = np.zeros((T, 1, K * BL), np.float32)
        m_next[:T - 1, 0, :] = mc.transpose(2, 0, 1).reshape(T, K * BL)[1:]
        mrep_c = np.broadcast_to(m_next, (T, H, K * BL))
        im = dict(shared)
        im['xm'] = xm_c.astype(nbf)
        im['mrep'] = np.ascontiguousarray(mrep_c).astype(nbf)
        in_maps.append(im)
    return in_maps


def kernel(**inputs):
    global LAST_RESULTS
    if 'nc' not in _CACHE:
        _CACHE['nc'] = _build()
    nc = _CACHE['nc']
    in_maps = _prep_inputs(inputs)
    trace = os.environ.get('BASS_TRACE', '') not in ('', '0')
    try:
        res = run_bass_kernel_spmd(nc, in_maps, list(range(NCORES)),
                                   trace=trace)
    except ModuleNotFoundError:
        # tracing needs the NTFF profile hook (absent in some images);
        # fall back to an untraced run
        os.environ['BASS_NEVER_TRACE'] = '1'
        res = run_bass_kernel_spmd(nc, in_maps, list(range(NCORES)),
                                   trace=False)
    LAST_RESULTS = res
    bout = np.asarray(inputs['bout'], np.float32)
    logits = np.empty((B, T, A), np.float32)
    hTf = np.empty((B, H), np.float32)
    for c in range(NCORES):
        bc = slice(c * BL, (c + 1) * BL)
        lgc = np.asarray(res.results[c]['lg'], np.float32)       # [A, T*BL]
        logits[bc] = lgc.reshape(A, T, BL).transpose(2, 1, 0) + bout
        hTf[bc] = np.asarray(res.results[c]['hT'], np.float32).T
    return logits, hTf


# revision 28
# speedup vs baseline: 1.0408x; 1.0408x over previous
"""Trainium2 Bass kernel for BanditSwitchRNN (mixture-of-experts GRU).

Sharding: data-parallel over batch B=1024 -> 8 cores x 128 batch each.
Per-core state h kept transposed in SBUF as [H=128 partitions, 128 batch cols].

Per-step math (expert selection is one-hot over K=4, known from x alone):
  - selection folded into inputs on host: xm[t] = masked x (+ mask rows for
    biases), mrep[t] = per-expert masks replicated over the 128 partitions.
  - h-side: hm[:, k*128:(k+1)*128] = h * mask_k  (4 DVE mults), then per gate
    4 accumulating matmuls lhsT=Wh_g[k].T, rhs=hm_k into one PSUM bank, with
    the gate's x-side matmul (lhsT=wxm block, rhs=xm[t]) opening the bank.
  - gates: r,z = sigmoid (ACT), n = tanh(Pnx + r*Pnh), h' = n + z*(h-n).
  - logits deferred: every h_t stored in a big SBUF buffer; one bulk matmul
    pass with Wout.T after the scan.
All matmul operands bf16, PSUM accumulation fp32 (validated 5.6e-3 rel err).
"""
import sys, os
sys.path.insert(0, '/opt/trn_rl_repo')
from contextlib import ExitStack

import numpy as np
import ml_dtypes

import concourse.bass as bass
import concourse.bacc as bacc
import concourse.mybir as mybir
import concourse.tile as tile
from concourse.bass_utils import run_bass_kernel_spmd

B, T, D, H, K, A = 1024, 256, 3, 128, 4, 2
NCORES = 8
BL = B // NCORES            # 128 batch per core
XR = 16                     # xm contraction rows: 12 masked-x + 4 mask (bias)
BF16 = mybir.dt.bfloat16
F32 = mybir.dt.float32
nbf = ml_dtypes.bfloat16
SIG = mybir.ActivationFunctionType.Sigmoid
TANH = mybir.ActivationFunctionType.Tanh

_CACHE = {}
LAST_RESULTS = None


def _build():
    nc = bacc.Bacc(None)
    whT = nc.declare_dram_parameter("whT", [3 * K, H, H], BF16, isOutput=False)
    wxm = nc.declare_dram_parameter("wxm", [4, XR, H], BF16, isOutput=False)
    wout = nc.declare_dram_parameter("wout", [H, A], BF16, isOutput=False)
    xm = nc.declare_dram_parameter("xm", [T, XR, BL], BF16, isOutput=False)
    mrep = nc.declare_dram_parameter("mrep", [T, H, K * BL], BF16, isOutput=False)
    lg = nc.declare_dram_parameter("lg", [A, T * BL], BF16, isOutput=True)
    hT = nc.declare_dram_parameter("hT", [H, BL], BF16, isOutput=True)

    with tile.TileContext(nc) as tc, ExitStack() as ctx:
        consts = ctx.enter_context(tc.tile_pool(name="consts", bufs=1))
        hallp = ctx.enter_context(tc.tile_pool(name="hall", bufs=1))

        whT_sb = []
        for i in range(3 * K):
            t_ = consts.tile([H, H], BF16, tag=f"whT{i}")
            nc.gpsimd.dma_start(t_[:], whT[i])
            whT_sb.append(t_)
        wxm_sb = []
        for i in range(4):
            t_ = consts.tile([XR, H], BF16, tag=f"wxm{i}")
            nc.gpsimd.dma_start(t_[:], wxm[i])
            wxm_sb.append(t_)
        wout_sb = consts.tile([H, A], BF16, tag="wout")
        nc.gpsimd.dma_start(wout_sb[:], wout[:])

        # Hall slot s holds h after step s-1 (slot 0 = h0 = 0)
        Hall = hallp.tile([H, (T + 1) * BL], BF16, tag="Hall")
        nc.vector.memset(Hall[:, 0:BL], 0.0)

        with (
            tc.tile_pool(name="stream", bufs=6) as streamp,
            tc.tile_pool(name="work", bufs=3) as workp,
            tc.tile_pool(name="psum", bufs=2, space="PSUM") as psump,
        ):
            hm = workp.tile([H, K * BL], BF16, tag="hm", name="hm")
            nc.vector.memset(hm[:], 0.0)
            for t in range(T):
                xm_t = streamp.tile([XR, BL], BF16, tag="xm")
                nc.gpsimd.dma_start(xm_t[:], xm[t])
                mr_t = streamp.tile([H, K * BL], BF16, tag="mrep")
                nc.gpsimd.dma_start(mr_t[:], mrep[t])

                Pr = psump.tile([H, BL], F32, tag="Pr")
                Pz = psump.tile([H, BL], F32, tag="Pz")
                Pnx = psump.tile([H, BL], F32, tag="Pnx")
                Pnh = psump.tile([H, BL], F32, tag="Pnh")

                # x-matmuls open each gate's accumulation group
                for q, (P, has_h) in enumerate(
                        ((Pr, True), (Pz, True), (Pnx, False), (Pnh, True))):
                    nc.tensor.matmul(P, wxm_sb[q][:], xm_t[:],
                                     start=True, stop=not has_h)
                # h-side: r, z first (sigmoid heads the tail), then nh
                for P, g in ((Pr, 0), (Pz, 1), (Pnh, 2)):
                    for k in range(K):
                        nc.tensor.matmul(P, whT_sb[g * K + k][:],
                                         hm[:, k * BL:(k + 1) * BL],
                                         start=False, stop=(k == K - 1))

                r_sb = workp.tile([H, BL], BF16, tag="r")
                nc.scalar.activation(r_sb[:], Pr[:], SIG)
                z_sb = workp.tile([H, BL], BF16, tag="z")
                nc.scalar.activation(z_sb[:], Pz[:], SIG)
                t1 = workp.tile([H, BL], F32, tag="t1")
                nc.vector.tensor_mul(t1[:], r_sb[:], Pnh[:])
                t2 = workp.tile([H, BL], F32, tag="t2")
                nc.vector.tensor_add(t2[:], t1[:], Pnx[:])
                n_sb = workp.tile([H, BL], BF16, tag="n")
                nc.scalar.activation(n_sb[:], t2[:], TANH)

                h_prev = Hall[:, t * BL:(t + 1) * BL]
                h_new = Hall[:, (t + 1) * BL:(t + 2) * BL]
                d_sb = workp.tile([H, BL], BF16, tag="d")
                nc.vector.tensor_sub(d_sb[:], h_prev, n_sb[:])
                e_sb = workp.tile([H, BL], BF16, tag="e")
                nc.vector.tensor_mul(e_sb[:], z_sb[:], d_sb[:])
                nc.vector.tensor_add(h_new, n_sb[:], e_sb[:])

                hm = workp.tile([H, K * BL], BF16, tag="hm", name="hm")
                for k in range(K):
                    nc.vector.tensor_mul(hm[:, k * BL:(k + 1) * BL], h_new,
                                         mr_t[:, k * BL:(k + 1) * BL])

        # logits end-pass: lg[a, t*BL+b] = sum_i Wout[a,i] * Hall[i, slot t+1]
        with (
            tc.tile_pool(name="lgp", bufs=4, space="PSUM") as lgp,
            tc.tile_pool(name="lgs", bufs=4) as lgsp,
        ):
            CH = 512
            for ci in range((T * BL) // CH):
                ps = lgp.tile([A, CH], F32, tag="lgps")
                nc.tensor.matmul(ps[:], wout_sb[:],
                                 Hall[:, BL + ci * CH:BL + (ci + 1) * CH],
                                 start=True, stop=True)
                st = lgsp.tile([A, CH], BF16, tag="lgsb")
                if ci % 2 == 0:
                    nc.scalar.copy(st[:], ps[:])
                else:
                    nc.vector.tensor_copy(st[:], ps[:])
                nc.gpsimd.dma_start(lg[:, ci * CH:(ci + 1) * CH], st[:])
            nc.gpsimd.dma_start(hT[:], Hall[:, T * BL:(T + 1) * BL])
    if not nc.is_finalized():
        nc.finalize()
    return nc


def _prep_inputs(inputs):
    x = np.asarray(inputs['x'], np.float32)
    a_sel = np.argmax(x[:, :, :2], axis=-1)
    r_sel = (x[:, :, 2] > 0.5).astype(np.int64)
    k_sel = a_sel + 2 * r_sel                                   # [B, T]
    mask = (k_sel[None] == np.arange(K)[:, None, None]).astype(np.float32)

    Wh = [np.asarray(inputs['Wh' + g], np.float32) for g in 'rzn']
    Wx = [np.asarray(inputs['Wx' + g], np.float32) for g in 'rzn']
    bx = [np.asarray(inputs['bx' + g], np.float32) for g in 'rzn']
    bh = {g: np.asarray(inputs['b' + g + 'h'], np.float32) for g in 'rzn'}

    whT = np.empty((3 * K, H, H), np.float32)
    for g in range(3):
        for k in range(K):
            whT[g * K + k] = Wh[g][k].T
    wxm = np.zeros((4, XR, H), np.float32)
    for bi, (Wxg, bsum) in enumerate((
            (Wx[0], bx[0] + bh['r']), (Wx[1], bx[1] + bh['z']),
            (Wx[2], bx[2]), (None, bh['n']))):
        if Wxg is not None:
            for k in range(K):
                for d in range(D):
                    wxm[bi, k * D + d] = Wxg[k, :, d]
        for k in range(K):
            wxm[bi, 3 * K + k] = bsum[k]
    wout = np.asarray(inputs['Wout'], np.float32).T             # [H, A]

    shared = {'whT': whT.astype(nbf), 'wxm': wxm.astype(nbf),
              'wout': wout.astype(nbf)}
    in_maps = []
    for c in range(NCORES):
        bc = slice(c * BL, (c + 1) * BL)
        mc = mask[:, bc, :]                                     # [K, BL, T]
        xc = x[bc]                                              # [BL, T, D]
        xm_c = np.zeros((T, XR, BL), np.float32)
        for k in range(K):
            for d in range(D):
                xm_c[:, k * D + d, :] = (mc[k] * xc[:, :, d]).T
            xm_c[:, 3 * K + k, :] = mc[k].T
        # mrep[t] = mask of step t+1 (hm built at end of step t feeds t+1),
        # replicated over the j partitions; last slot unused -> zeros.
        m_next = np.zeros((T, 1, K * BL), np.float32)
        m_next[:T - 1, 0, :] = mc.transpose(2, 0, 1).reshape(T, K * BL)[1:]
        mrep_c = np.broadcast_to(m_next, (T, H, K * BL))
        im = dict(shared)
        im['xm'] = xm_c.astype(nbf)
        im['mrep'] = np.ascontiguousarray(mrep_c).astype(nbf)
        in_maps.append(im)
    return in_maps


def kernel(**inputs):
    global LAST_RESULTS
    if 'nc' not in _CACHE:
        _CACHE['nc'] = _build()
    nc = _CACHE['nc']
    in_maps = _prep_inputs(inputs)
    trace = os.environ.get('BASS_TRACE', '') not in ('', '0')
    try:
        res = run_bass_kernel_spmd(nc, in_maps, list(range(NCORES)),
                                   trace=trace)
    except ModuleNotFoundError:
        # tracing needs the NTFF profile hook (absent in some images);
        # fall back to an untraced run
        os.environ['BASS_NEVER_TRACE'] = '1'
        res = run_bass_kernel_spmd(nc, in_maps, list(range(NCORES)),
                                   trace=False)
    LAST_RESULTS = res
    bout = np.asarray(inputs['bout'], np.float32)
    logits = np.empty((B, T, A), np.float32)
    hTf = np.empty((B, H), np.float32)
    for c in range(NCORES):
        bc = slice(c * BL, (c + 1) * BL)
        lgc = np.asarray(res.results[c]['lg'], np.float32)       # [A, T*BL]
        logits[bc] = lgc.reshape(A, T, BL).transpose(2, 1, 0) + bout
        hTf[bc] = np.asarray(res.results[c]['hT'], np.float32).T
    return logits, hTf
